# revision 4
# baseline (speedup 1.0000x reference)
"""Sliding-window (W=128) multi-head attention block for Trainium2, 8 cores.

Reference computation (B=2, T=2048, E=1024, H=16, D=64, W=128):
    qkv = x @ w_qkv.T ; split q,k,v ; heads ; att = softmax(mask(q k^T / 8)) v
    out = att_concat @ w_out.T

Sharding: data-parallel over B (2) x tensor-parallel over head groups (4),
so each of the 8 cores handles (one batch, 4 heads).  The output projection
is computed per-core against the 256 w_out columns belonging to its heads,
giving a partial [T, E] output (bf16); the host sums the 4 partials per
batch in f32.

Attention is computed in "transposed score" form to avoid PE transposes:
    S^T[k, q] = k^T.T @ q^T   (keys on partitions, d=64 contraction)
    E^T = exp(S^T)            (ACT, psum->sbuf bf16)
    mask -> 0 via affine_select on GpSimd (band structure is affine)
    [O'^T ; l] = [V | 1].T @ E^T  (l = softmax denominator, folded into the
                                   O matmul via ones-columns in the weights)
    attT = O'^T * (1/l)       (DVE reciprocal + multiply, psum in)
The two heads of a q^T/k^T pair chunk live at partitions 0:64 / 64:128 and
their K=64 S^T matmuls are row-tiled (tile_position auto-derived from the
base partition), so the pair runs concurrently on the PE array.

The 1/sqrt(D) scale is folded into the q weights on the host.
"""

import numpy as np
import ml_dtypes

import concourse.bass as bass
import concourse.bacc as bacc
import concourse.mybir as mybir
import concourse.tile as tile
from concourse.bass_utils import run_bass_kernel_spmd

B, T, E, H, W = 2, 2048, 1024, 16, 128
D = E // H            # 64
HPC = 4               # heads per core
N_CORES = 8
SCALE = 1.0 / float(np.sqrt(D))

BF16 = mybir.dt.bfloat16
F32 = mybir.dt.float32

KO = E // 128         # 8 contraction chunks
NQT = T // 128        # 16 query tiles
NT512 = T // 512      # 4 tiles for the projections

# Row-tiled K=64 S^T matmuls (pair-concurrent).  False falls back to
# zero-padded K=128 per-head kT (no partition-offset PE operands).
ROWPACK = True


def build_bass():
    nc = bacc.Bacc()
    xT = nc.declare_dram_parameter("xT", [E, T], BF16, isOutput=False)
    wqk = nc.declare_dram_parameter("wqk", [E, 2 * HPC * D], BF16, isOutput=False)
    wv = nc.declare_dram_parameter("wv", [E, HPC * D], BF16, isOutput=False)
    wout = nc.declare_dram_parameter("wout", [HPC * D, E], BF16, isOutput=False)
    outp = nc.declare_dram_parameter("outp", [T, E], BF16, isOutput=True)

    with tile.TileContext(nc) as tc:
        with (
            tc.tile_pool(name="persist", bufs=1) as persist,
            tc.tile_pool(name="work", bufs=3) as work,
            tc.tile_pool(name="rlp", bufs=2) as rlp,
            tc.tile_pool(name="outw", bufs=3) as outw,
            tc.tile_pool(name="ps_mm", bufs=2, space="PSUM") as ps_mm,
            tc.tile_pool(name="ps_s", bufs=2, space="PSUM") as ps_s,
            tc.tile_pool(name="ps_o", bufs=2, space="PSUM") as ps_o,
        ):
            # ---- persistent tiles ----
            wqk_sb = persist.tile([128, KO, 2 * HPC * D], BF16)
            wv_sb = persist.tile([128, KO, HPC * D], BF16)
            wout_sb = persist.tile([128, 2, E], BF16)
            xT_sb = persist.tile([128, KO, T], BF16)
            qkT_sb = persist.tile([128, 2, T], BF16)   # q^T pairs (scaled)
            if ROWPACK:
                kT_sb = persist.tile([128, 2, T], BF16)  # k^T pairs
            else:
                kT_sb = persist.tile([128, HPC, T], BF16)  # zero-padded k^T
                nc.vector.memset(kT_sb, 0.0)
            # V augmented with ones columns: [:, kt, h, 0:64]=V_h, 64:128=1
            vA_sb = persist.tile([128, NQT, HPC, 128], BF16)
            nc.gpsimd.memset(vA_sb[:, :, :, D:128], 1.0)
            attT_sb = persist.tile([128, 2, T], BF16)  # O^T, chunk j: heads 2j,2j+1

            # ---- input DMAs (order matters: ti=0 needs wqk + x0 first) ----
            nc.sync.dma_start(
                out=wqk_sb, in_=wqk[:, :].rearrange("(ko p) m -> p ko m", p=128))
            x_ap = xT[:, :].rearrange("(ko p) t -> p ko t", p=128)
            nc.sync.dma_start(out=xT_sb[:, :, 0:512], in_=x_ap[:, :, 0:512])
            nc.sync.dma_start(
                out=wv_sb, in_=wv[:, :].rearrange("(ko p) m -> p ko m", p=128))
            nc.sync.dma_start(
                out=wout_sb, in_=wout[:, :].rearrange("(c p) m -> p c m", p=128))
            for ti in range(1, NT512):
                tsl = slice(ti * 512, (ti + 1) * 512)
                nc.sync.dma_start(out=xT_sb[:, :, tsl], in_=x_ap[:, :, tsl])

            def stage1(ti):
                tsl = slice(ti * 512, (ti + 1) * 512)
                for mi in range(4):
                    ps = ps_mm.tile([128, 512], F32, tag="mm")
                    for ko in range(KO):
                        nc.tensor.matmul(
                            ps,
                            lhsT=wqk_sb[:, ko, mi * 128:(mi + 1) * 128],
                            rhs=xT_sb[:, ko, tsl],
                            start=(ko == 0), stop=(ko == KO - 1),
                        )
                    if mi < 2:
                        nc.vector.tensor_copy(out=qkT_sb[:, mi, tsl], in_=ps)
                    elif ROWPACK:
                        nc.scalar.copy(out=kT_sb[:, mi - 2, tsl], in_=ps)
                    else:
                        hp = (mi - 2) * 2
                        nc.scalar.copy(out=kT_sb[0:64, hp, tsl], in_=ps[0:64])
                        nc.scalar.copy(
                            out=kT_sb[64:128, hp + 1, tsl], in_=ps[64:128])
                for j in range(4):
                    t0 = ti * 512 + j * 128
                    ps = ps_mm.tile([128, 512], F32, tag="mm")
                    for ko in range(KO):
                        nc.tensor.matmul(
                            ps[:, 0:HPC * D],
                            lhsT=xT_sb[:, ko, t0:t0 + 128],
                            rhs=wv_sb[:, ko, :],
                            start=(ko == 0), stop=(ko == KO - 1),
                        )
                    nc.vector.tensor_copy(
                        out=vA_sb[:, ti * 4 + j, :, 0:D],
                        in_=ps[:, 0:HPC * D])

            def emit_S(qi):
                """S^T matmuls + exp + band mask; returns E^T sbuf tile
                laid out [128 k, par, ci, mi, q] (head h = 2*mi + par)."""
                qsl = slice(qi * 128, (qi + 1) * 128)
                cis = [1] if qi == 0 else [0, 1]
                psS = [ps_s.tile([128, 2, 2, 128], F32, tag=f"S{par}",
                                 name=f"psS{par}")
                       for par in range(2)]
                for ci in cis:
                    kt = qi - 1 + ci
                    ksl = slice(kt * 128, (kt + 1) * 128)
                    for mi in range(2):
                        for par in range(2):
                            if ROWPACK:
                                prows = slice(par * 64, par * 64 + 64)
                                lhsT = kT_sb[prows, mi, ksl]
                                rhs = qkT_sb[prows, mi, qsl]
                            else:
                                lhsT = kT_sb[:, 2 * mi + par, ksl]
                                rhs = qkT_sb[:, mi, qsl]
                            nc.tensor.matmul(
                                psS[par][:, ci, mi, :], lhsT=lhsT, rhs=rhs,
                                start=True, stop=True,
                            )
                esb = work.tile([128, 2, 2, 2, 128], BF16, tag="E")
                for par in range(2):
                    if qi == 0:
                        nc.scalar.activation(
                            out=esb[:, par, 1, :, :], in_=psS[par][:, 1, :, :],
                            func=mybir.ActivationFunctionType.Exp)
                    else:
                        nc.scalar.activation(
                            out=esb[:, par, :, :, :], in_=psS[par],
                            func=mybir.ActivationFunctionType.Exp)
                if qi == 0:
                    nc.gpsimd.memset(esb[:, :, 0, :, :], 0.0)
                # band mask -> zero.  esb free dims in the selected slice are
                # (par, mi, q); partition is k (local to the key tile).
                # ci=1 (kt==qi): keep iff q - k >= 0
                nc.gpsimd.affine_select(
                    out=esb[:, :, 1, :, :], in_=esb[:, :, 1, :, :],
                    compare_op=mybir.AluOpType.is_ge, fill=0.0,
                    base=0, channel_multiplier=-1,
                    pattern=[[0, 2], [0, 2], [1, 128]],
                )
                if qi > 0:
                    # ci=0 (kt==qi-1): keep iff k - q - 1 >= 0
                    nc.gpsimd.affine_select(
                        out=esb[:, :, 0, :, :], in_=esb[:, :, 0, :, :],
                        compare_op=mybir.AluOpType.is_ge, fill=0.0,
                        base=-1, channel_multiplier=1,
                        pattern=[[0, 2], [0, 2], [-1, 128]],
                    )
                return esb

            def emit_O(qi, esb):
                """[O'^T ; l] matmuls, then attT = O'^T / l."""
                qsl = slice(qi * 128, (qi + 1) * 128)
                cis = [1] if qi == 0 else [0, 1]
                # col blocks ordered [h0, h2, h1, h3] so each parity's pair
                # of heads is contiguous for the normalize ops below
                psO = ps_o.tile([128, HPC, 128], F32, tag="O")
                for h in range(HPC):
                    mi, par = h // 2, h % 2
                    blk = par * 2 + mi
                    for i, ci in enumerate(cis):
                        kt = qi - 1 + ci
                        nc.tensor.matmul(
                            psO[:, blk, :],
                            lhsT=vA_sb[:, kt, h, :],
                            rhs=esb[:, par, ci, mi, :],
                            start=(i == 0), stop=(i == len(cis) - 1),
                        )
                rl = rlp.tile([64, HPC, 128], F32, tag="rl")
                nc.vector.reciprocal(out=rl, in_=psO[64:128, :, :])
                for s in range(2):
                    nc.vector.tensor_tensor(
                        attT_sb[s * 64:s * 64 + 64, :, qsl],
                        psO[0:64, 2 * s:2 * s + 2, :],
                        rl[:, 2 * s:2 * s + 2, :],
                        mybir.AluOpType.mult,
                    )

            def stage3(qi):
                tsl = slice(qi * 128, (qi + 1) * 128)
                o_sb = outw.tile([128, E], BF16, tag="osb")
                for nh in range(2):
                    po = ps_mm.tile([128, 512], F32, tag="mm")
                    for j in range(2):
                        nc.tensor.matmul(
                            po,
                            lhsT=attT_sb[:, j, tsl],
                            rhs=wout_sb[:, j, nh * 512:(nh + 1) * 512],
                            start=(j == 0), stop=(j == 1),
                        )
                    if nh == 0:
                        nc.vector.tensor_copy(out=o_sb[:, 0:512], in_=po)
                    else:
                        nc.scalar.copy(out=o_sb[:, 512:1024], in_=po)
                nc.sync.dma_start(out=outp[tsl, :], in_=o_sb)

            # ---- software-pipelined main loop (O/stage3 lag one qi behind
            # S/exp/mask so the PE never waits on ACT/GpSimd) ----
            prev = None
            for ti in range(NT512):
                stage1(ti)
                for qi in range(4 * ti, 4 * ti + 4):
                    esb = emit_S(qi)
                    if prev is not None:
                        emit_O(*prev)
                        stage3(prev[0])
                    prev = (qi, esb)
            emit_O(*prev)
            stage3(prev[0])

    nc.finalize()
    return nc


_NC_CACHE = None


def _get_nc():
    global _NC_CACHE
    if _NC_CACHE is None:
        _NC_CACHE = build_bass()
    return _NC_CACHE


def make_in_maps(x, w_qkv, w_out):
    x = np.asarray(x, dtype=np.float32)
    w_qkv = np.asarray(w_qkv, dtype=np.float32)
    w_out = np.asarray(w_out, dtype=np.float32)
    bf = ml_dtypes.bfloat16
    in_maps = []
    for c in range(N_CORES):
        b = c // 4
        hs = (c % 4) * HPC
        rows = slice(hs * D, (hs + HPC) * D)
        wq = w_qkv[0 * E:, :][rows] * SCALE    # fold 1/sqrt(D) into q
        wk = w_qkv[1 * E:, :][rows]
        wvs = w_qkv[2 * E:, :][rows]
        in_maps.append({
            "xT": np.ascontiguousarray(x[b].T).astype(bf),
            "wqk": np.ascontiguousarray(
                np.concatenate([wq, wk], axis=0).T).astype(bf),
            "wv": np.ascontiguousarray(wvs.T).astype(bf),
            "wout": np.ascontiguousarray(w_out[:, rows].T).astype(bf),
        })
    return in_maps


def run(x, w_qkv, w_out, **spmd_kwargs):
    nc = _get_nc()
    in_maps = make_in_maps(x, w_qkv, w_out)
    res = run_bass_kernel_spmd(nc, in_maps, core_ids=list(range(N_CORES)),
                               **spmd_kwargs)
    outs = [r["outp"] for r in res.results]
    out = np.empty((B, T, E), dtype=np.float32)
    for b in range(B):
        acc = outs[4 * b].astype(np.float32)
        for c in range(4 * b + 1, 4 * b + 4):
            acc = acc + outs[c].astype(np.float32)
        out[b] = acc
    return out, res


def kernel(x, w_qkv, w_out):
    out, _ = run(x, w_qkv, w_out)
    return out


# revision 9
# speedup vs baseline: 1.3411x; 1.3411x over previous
"""Sliding-window (W=128) multi-head attention block for Trainium2, 8 cores.

Reference computation (B=2, T=2048, E=1024, H=16, D=64, W=128):
    qkv = x @ w_qkv.T ; split q,k,v ; heads ; att = softmax(mask(q k^T / 8)) v
    out = att_concat @ w_out.T

Sharding: data-parallel over B (2) x tensor-parallel over head groups (4),
so each of the 8 cores handles (one batch, 4 heads).  The output projection
is computed per-core against the 256 w_out columns belonging to its heads,
giving a partial [T, E] output (bf16); the host sums the 4 partials per
batch in f32.

Attention is computed in "transposed score" form to avoid PE transposes:
    S^T[k, q] = k^T.T @ q^T   (keys on partitions, d=64 contraction)
    E^T = exp(S^T)            (ACT, psum->sbuf bf16)
    mask -> 0 via affine_select on GpSimd (band structure is affine)
    [O'^T ; l] = [V | 1].T @ E^T  (l = softmax denominator, folded into the
                                   O matmul via ones-columns in the weights)
    attT = O'^T * (1/l)       (DVE reciprocal + multiply, psum in)
The two heads of a q^T/k^T pair chunk live at partitions 0:64 / 64:128 and
their K=64 S^T matmuls are row-tiled (tile_position auto-derived from the
base partition), so the pair runs concurrently on the PE array.

The 1/sqrt(D) scale is folded into the q weights on the host.
"""

import numpy as np
import ml_dtypes

import concourse.bass as bass
import concourse.bacc as bacc
import concourse.mybir as mybir
import concourse.tile as tile
from concourse.bass_utils import run_bass_kernel_spmd

B, T, E, H, W = 2, 2048, 1024, 16, 128
D = E // H            # 64
HPC = 4               # heads per core
N_CORES = 8
SCALE = 1.0 / float(np.sqrt(D))

BF16 = mybir.dt.bfloat16
F32 = mybir.dt.float32

KO = E // 128         # 8 contraction chunks
NQT = T // 128        # 16 query tiles
NT512 = T // 512      # 4 tiles for the projections

# Row-tiled K=64 S^T matmuls (pair-concurrent).  False falls back to
# zero-padded K=128 per-head kT (no partition-offset PE operands).
ROWPACK = True


def build_bass():
    nc = bacc.Bacc()
    xT = nc.declare_dram_parameter("xT", [E, T], BF16, isOutput=False)
    wqk = nc.declare_dram_parameter("wqk", [E, 2 * HPC * D], BF16, isOutput=False)
    wv = nc.declare_dram_parameter("wv", [E, HPC * D], BF16, isOutput=False)
    wout = nc.declare_dram_parameter("wout", [HPC * D, E], BF16, isOutput=False)
    outp = nc.declare_dram_parameter("outp", [T, E], BF16, isOutput=True)

    with tile.TileContext(nc) as tc:
        with (
            tc.tile_pool(name="persist", bufs=1) as persist,
            tc.tile_pool(name="work", bufs=3) as work,
            tc.tile_pool(name="rlp", bufs=2) as rlp,
            tc.tile_pool(name="outw", bufs=3) as outw,
            tc.tile_pool(name="ps_mm", bufs=2, space="PSUM") as ps_mm,
            tc.tile_pool(name="ps_s", bufs=2, space="PSUM") as ps_s,
            tc.tile_pool(name="ps_o", bufs=2, space="PSUM") as ps_o,
        ):
            # ---- persistent tiles ----
            wqk_sb = persist.tile([128, KO, 2 * HPC * D], BF16)
            wv_sb = persist.tile([128, KO, HPC * D], BF16)
            wout_sb = persist.tile([128, 2, E], BF16)
            xT_sb = persist.tile([128, KO, T], BF16)
            qkT_sb = persist.tile([128, 2, T], BF16)   # q^T pairs (scaled)
            if ROWPACK:
                kT_sb = persist.tile([128, 2, T], BF16)  # k^T pairs
            else:
                kT_sb = persist.tile([128, HPC, T], BF16)  # zero-padded k^T
                nc.vector.memset(kT_sb, 0.0)
            # V augmented with ones columns: [:, kt, h, 0:64]=V_h, 64:128=1
            vA_sb = persist.tile([128, NQT, HPC, 128], BF16)
            nc.gpsimd.memset(vA_sb[:, :, :, D:128], 1.0)
            attT_sb = persist.tile([128, 2, T], BF16)  # O^T, chunk j: heads 2j,2j+1

            # ---- PE warmup: ~16 dependency-free matmuls release the HAM
            # clock gate (K=4/8 -> 8/8 needs ~3.4us of sustained PE activity)
            # while the first input DMAs are still in flight ----
            wu_sb = persist.tile([128, 512], BF16)
            nc.vector.memset(wu_sb, 0.0)
            for wi in range(16):
                wu_ps = ps_o.tile([128, HPC, 128], F32, tag="O")
                nc.tensor.matmul(
                    wu_ps.rearrange("p a b -> p (a b)"), lhsT=wu_sb[:, 0:128],
                    rhs=wu_sb, start=True, stop=True)

            # ---- input DMAs (order matters: ti=0 needs wqk + x0 first) ----
            nc.sync.dma_start(
                out=wqk_sb, in_=wqk[:, :].rearrange("(ko p) m -> p ko m", p=128))
            x_ap = xT[:, :].rearrange("(ko p) t -> p ko t", p=128)
            nc.sync.dma_start(out=xT_sb[:, :, 0:512], in_=x_ap[:, :, 0:512])
            nc.sync.dma_start(
                out=wv_sb, in_=wv[:, :].rearrange("(ko p) m -> p ko m", p=128))
            nc.sync.dma_start(
                out=wout_sb, in_=wout[:, :].rearrange("(c p) m -> p c m", p=128))
            for ti in range(1, NT512):
                tsl = slice(ti * 512, (ti + 1) * 512)
                nc.sync.dma_start(out=xT_sb[:, :, tsl], in_=x_ap[:, :, tsl])

            def stage1(ti):
                tsl = slice(ti * 512, (ti + 1) * 512)
                for mi in range(4):
                    ps = ps_mm.tile([128, 512], F32, tag="mm")
                    for ko in range(KO):
                        nc.tensor.matmul(
                            ps,
                            lhsT=wqk_sb[:, ko, mi * 128:(mi + 1) * 128],
                            rhs=xT_sb[:, ko, tsl],
                            start=(ko == 0), stop=(ko == KO - 1),
                        )
                    if mi < 2:
                        nc.vector.tensor_copy(out=qkT_sb[:, mi, tsl], in_=ps)
                    elif ROWPACK:
                        nc.scalar.copy(out=kT_sb[:, mi - 2, tsl], in_=ps)
                    else:
                        hp = (mi - 2) * 2
                        nc.scalar.copy(out=kT_sb[0:64, hp, tsl], in_=ps[0:64])
                        nc.scalar.copy(
                            out=kT_sb[64:128, hp + 1, tsl], in_=ps[64:128])
                for j in range(4):
                    t0 = ti * 512 + j * 128
                    ps = ps_mm.tile([128, 512], F32, tag="mm")
                    for ko in range(KO):
                        nc.tensor.matmul(
                            ps[:, 0:HPC * D],
                            lhsT=xT_sb[:, ko, t0:t0 + 128],
                            rhs=wv_sb[:, ko, :],
                            start=(ko == 0), stop=(ko == KO - 1),
                        )
                    nc.vector.tensor_copy(
                        out=vA_sb[:, ti * 4 + j, :, 0:D],
                        in_=ps[:, 0:HPC * D])

            def emit_S(qi):
                """S^T matmuls + exp + band mask; returns E^T sbuf tile
                laid out [128 k, par, ci, mi, q] (head h = 2*mi + par)."""
                qsl = slice(qi * 128, (qi + 1) * 128)
                cis = [1] if qi == 0 else [0, 1]
                psS = [ps_s.tile([128, 2, 2, 128], F32, tag=f"S{par}",
                                 name=f"psS{par}")
                       for par in range(2)]
                for ci in cis:
                    kt = qi - 1 + ci
                    ksl = slice(kt * 128, (kt + 1) * 128)
                    for mi in range(2):
                        for par in range(2):
                            if ROWPACK:
                                prows = slice(par * 64, par * 64 + 64)
                                lhsT = kT_sb[prows, mi, ksl]
                                rhs = qkT_sb[prows, mi, qsl]
                            else:
                                lhsT = kT_sb[:, 2 * mi + par, ksl]
                                rhs = qkT_sb[:, mi, qsl]
                            nc.tensor.matmul(
                                psS[par][:, ci, mi, :], lhsT=lhsT, rhs=rhs,
                                start=True, stop=True,
                            )
                esb = work.tile([128, 2, 2, 2, 128], BF16, tag="E")
                for par in range(2):
                    if qi == 0:
                        nc.scalar.activation(
                            out=esb[:, par, 1, :, :], in_=psS[par][:, 1, :, :],
                            func=mybir.ActivationFunctionType.Exp)
                    else:
                        nc.scalar.activation(
                            out=esb[:, par, :, :, :], in_=psS[par],
                            func=mybir.ActivationFunctionType.Exp)
                if qi == 0:
                    nc.gpsimd.memset(esb[:, :, 0, :, :], 0.0)
                # band mask -> zero.  esb free dims in the selected slice are
                # (par, mi, q); partition is k (local to the key tile).
                # ci=1 (kt==qi): keep iff q - k >= 0
                nc.gpsimd.affine_select(
                    out=esb[:, :, 1, :, :], in_=esb[:, :, 1, :, :],
                    compare_op=mybir.AluOpType.is_ge, fill=0.0,
                    base=0, channel_multiplier=-1,
                    pattern=[[0, 2], [0, 2], [1, 128]],
                )
                if qi > 0:
                    # ci=0 (kt==qi-1): keep iff k - q - 1 >= 0
                    nc.gpsimd.affine_select(
                        out=esb[:, :, 0, :, :], in_=esb[:, :, 0, :, :],
                        compare_op=mybir.AluOpType.is_ge, fill=0.0,
                        base=-1, channel_multiplier=1,
                        pattern=[[0, 2], [0, 2], [-1, 128]],
                    )
                return esb

            def emit_O(qi, esb):
                """[O'^T ; l] matmuls, then attT = O'^T / l."""
                qsl = slice(qi * 128, (qi + 1) * 128)
                cis = [1] if qi == 0 else [0, 1]
                # col blocks ordered [h0, h2, h1, h3] so each parity's pair
                # of heads is contiguous for the normalize ops below
                psO = ps_o.tile([128, HPC, 128], F32, tag="O")
                for h in range(HPC):
                    mi, par = h // 2, h % 2
                    blk = par * 2 + mi
                    for i, ci in enumerate(cis):
                        kt = qi - 1 + ci
                        nc.tensor.matmul(
                            psO[:, blk, :],
                            lhsT=vA_sb[:, kt, h, :],
                            rhs=esb[:, par, ci, mi, :],
                            start=(i == 0), stop=(i == len(cis) - 1),
                        )
                # stage l into SBUF first: the approx reciprocal's
                # BITWISE_NOT seed needs the IEEE bit pattern, which a PSUM
                # read does not reliably provide on hardware
                l_sb = rlp.tile([64, HPC, 128], F32, tag="lsb")
                nc.scalar.copy(out=l_sb, in_=psO[64:128, :, :])
                rl = rlp.tile([64, HPC, 128], F32, tag="rl")
                # l > 0 and well-scaled: far from the approx-fast edge cases;
                # ~18 correct bits vs the multi-pass exact reciprocal (5x cost)
                nc.vector.reciprocal_approx_fast(out=rl, in_=l_sb)
                for s in range(2):
                    nc.vector.tensor_tensor(
                        attT_sb[s * 64:s * 64 + 64, :, qsl],
                        psO[0:64, 2 * s:2 * s + 2, :],
                        rl[:, 2 * s:2 * s + 2, :],
                        mybir.AluOpType.mult,
                    )

            def stage3(qi):
                tsl = slice(qi * 128, (qi + 1) * 128)
                o_sb = outw.tile([128, E], BF16, tag="osb")
                for nh in range(2):
                    po = ps_mm.tile([128, 512], F32, tag="mm")
                    for j in range(2):
                        nc.tensor.matmul(
                            po,
                            lhsT=attT_sb[:, j, tsl],
                            rhs=wout_sb[:, j, nh * 512:(nh + 1) * 512],
                            start=(j == 0), stop=(j == 1),
                        )
                    if nh == 0:
                        nc.vector.tensor_copy(out=o_sb[:, 0:512], in_=po)
                    else:
                        nc.scalar.copy(out=o_sb[:, 512:1024], in_=po)
                nc.sync.dma_start(out=outp[tsl, :], in_=o_sb)

            # ---- software-pipelined main loop (O/stage3 lag one qi behind
            # S/exp/mask so the PE never waits on ACT/GpSimd) ----
            prev = None
            for ti in range(NT512):
                stage1(ti)
                for qi in range(4 * ti, 4 * ti + 4):
                    esb = emit_S(qi)
                    if prev is not None:
                        emit_O(*prev)
                        stage3(prev[0])
                    prev = (qi, esb)
            emit_O(*prev)
            stage3(prev[0])

    nc.finalize()
    return nc


_NC_CACHE = None


def _get_nc():
    global _NC_CACHE
    if _NC_CACHE is None:
        _NC_CACHE = build_bass()
    return _NC_CACHE


def make_in_maps(x, w_qkv, w_out):
    x = np.asarray(x, dtype=np.float32)
    w_qkv = np.asarray(w_qkv, dtype=np.float32)
    w_out = np.asarray(w_out, dtype=np.float32)
    bf = ml_dtypes.bfloat16
    in_maps = []
    for c in range(N_CORES):
        b = c // 4
        hs = (c % 4) * HPC
        rows = slice(hs * D, (hs + HPC) * D)
        wq = w_qkv[0 * E:, :][rows] * SCALE    # fold 1/sqrt(D) into q
        wk = w_qkv[1 * E:, :][rows]
        wvs = w_qkv[2 * E:, :][rows]
        in_maps.append({
            "xT": np.ascontiguousarray(x[b].T).astype(bf),
            "wqk": np.ascontiguousarray(
                np.concatenate([wq, wk], axis=0).T).astype(bf),
            "wv": np.ascontiguousarray(wvs.T).astype(bf),
            "wout": np.ascontiguousarray(w_out[:, rows].T).astype(bf),
        })
    return in_maps


def run(x, w_qkv, w_out, **spmd_kwargs):
    nc = _get_nc()
    in_maps = make_in_maps(x, w_qkv, w_out)
    res = run_bass_kernel_spmd(nc, in_maps, core_ids=list(range(N_CORES)),
                               **spmd_kwargs)
    outs = [r["outp"] for r in res.results]
    out = np.empty((B, T, E), dtype=np.float32)
    for b in range(B):
        acc = outs[4 * b].astype(np.float32)
        for c in range(4 * b + 1, 4 * b + 4):
            acc = acc + outs[c].astype(np.float32)
        out[b] = acc
    return out, res


def kernel(x, w_qkv, w_out):
    out, _ = run(x, w_qkv, w_out)
    return out


# revision 10
# speedup vs baseline: 1.5482x; 1.1544x over previous
"""Sliding-window (W=128) multi-head attention block for Trainium2, 8 cores.

Reference computation (B=2, T=2048, E=1024, H=16, D=64, W=128):
    qkv = x @ w_qkv.T ; split q,k,v ; heads ; att = softmax(mask(q k^T / 8)) v
    out = att_concat @ w_out.T

Sharding: data-parallel over B (2) x tensor-parallel over head groups (4),
so each of the 8 cores handles (one batch, 4 heads).  The output projection
is computed per-core against the 256 w_out columns belonging to its heads,
giving a partial [T, E] output (bf16); the host sums the 4 partials per
batch in f32.

Attention is computed in "transposed score" form to avoid PE transposes:
    S^T[k, q] = k^T.T @ q^T   (keys on partitions, d=64 contraction)
    E^T = exp(S^T)            (ACT, psum->sbuf bf16)
    mask -> 0 via affine_select on GpSimd (band structure is affine)
    [O'^T ; l] = [V | 1].T @ E^T  (l = softmax denominator, folded into the
                                   O matmul via ones-columns in the weights)
    attT = O'^T * (1/l)       (DVE reciprocal + multiply, psum in)
The two heads of a q^T/k^T pair chunk live at partitions 0:64 / 64:128 and
their K=64 S^T matmuls are row-tiled (tile_position auto-derived from the
base partition), so the pair runs concurrently on the PE array.

The 1/sqrt(D) scale is folded into the q weights on the host.
"""

import numpy as np
import ml_dtypes

import concourse.bass as bass
import concourse.bacc as bacc
import concourse.mybir as mybir
import concourse.tile as tile
from concourse.bass_utils import run_bass_kernel_spmd

B, T, E, H, W = 2, 2048, 1024, 16, 128
D = E // H            # 64
HPC = 4               # heads per core
N_CORES = 8
SCALE = 1.0 / float(np.sqrt(D))

BF16 = mybir.dt.bfloat16
F32 = mybir.dt.float32

KO = E // 128         # 8 contraction chunks
NQT = T // 128        # 16 query tiles
NT512 = T // 512      # 4 tiles for the projections

# Row-tiled K=64 S^T matmuls (pair-concurrent).  False falls back to
# zero-padded K=128 per-head kT (no partition-offset PE operands).
ROWPACK = True


def build_bass():
    nc = bacc.Bacc()
    xT = nc.declare_dram_parameter("xT", [E, T], BF16, isOutput=False)
    wqk = nc.declare_dram_parameter("wqk", [E, 2 * HPC * D], BF16, isOutput=False)
    wv = nc.declare_dram_parameter("wv", [E, HPC * D], BF16, isOutput=False)
    wout = nc.declare_dram_parameter("wout", [HPC * D, E], BF16, isOutput=False)
    outp = nc.declare_dram_parameter("outp", [T, E], BF16, isOutput=True)

    with tile.TileContext(nc) as tc:
        with (
            tc.tile_pool(name="persist", bufs=1) as persist,
            tc.tile_pool(name="work", bufs=3) as work,
            tc.tile_pool(name="rlp", bufs=2) as rlp,
            tc.tile_pool(name="outw", bufs=3) as outw,
            tc.tile_pool(name="ps_mm", bufs=2, space="PSUM") as ps_mm,
            tc.tile_pool(name="ps_s", bufs=2, space="PSUM") as ps_s,
            tc.tile_pool(name="ps_o", bufs=2, space="PSUM") as ps_o,
        ):
            # ---- persistent tiles ----
            wqk_sb = persist.tile([128, KO, 2 * HPC * D], BF16)
            wv_sb = persist.tile([128, KO, HPC * D], BF16)
            wout_sb = persist.tile([128, 2, E], BF16)
            xT_sb = persist.tile([128, KO, T], BF16)
            qkT_sb = persist.tile([128, 2, T], BF16)   # q^T pairs (scaled)
            if ROWPACK:
                kT_sb = persist.tile([128, 2, T], BF16)  # k^T pairs
            else:
                kT_sb = persist.tile([128, HPC, T], BF16)  # zero-padded k^T
                nc.vector.memset(kT_sb, 0.0)
            # V augmented with ones columns: [:, kt, h, 0:64]=V_h, 64:128=1
            vA_sb = persist.tile([128, NQT, HPC, 128], BF16)
            nc.gpsimd.memset(vA_sb[:, :, :, D:128], 1.0)
            attT_sb = persist.tile([128, 2, T], BF16)  # O^T, chunk j: heads 2j,2j+1

            # ---- PE warmup: ~16 dependency-free matmuls release the HAM
            # clock gate (K=4/8 -> 8/8 needs ~3.4us of sustained PE activity)
            # while the first input DMAs are still in flight ----
            wu_sb = persist.tile([128, 512], BF16)
            nc.vector.memset(wu_sb, 0.0)
            for wi in range(16):
                wu_ps = ps_o.tile([128, HPC, 128], F32, tag="O")
                nc.tensor.matmul(
                    wu_ps.rearrange("p a b -> p (a b)"), lhsT=wu_sb[:, 0:128],
                    rhs=wu_sb, start=True, stop=True)

            # ---- input DMAs (order matters: ti=0 needs wqk + x0 first) ----
            nc.sync.dma_start(
                out=wqk_sb, in_=wqk[:, :].rearrange("(ko p) m -> p ko m", p=128))
            x_ap = xT[:, :].rearrange("(ko p) t -> p ko t", p=128)
            nc.sync.dma_start(out=xT_sb[:, :, 0:512], in_=x_ap[:, :, 0:512])
            nc.sync.dma_start(
                out=wv_sb, in_=wv[:, :].rearrange("(ko p) m -> p ko m", p=128))
            nc.sync.dma_start(
                out=wout_sb, in_=wout[:, :].rearrange("(c p) m -> p c m", p=128))
            for ti in range(1, NT512):
                tsl = slice(ti * 512, (ti + 1) * 512)
                nc.sync.dma_start(out=xT_sb[:, :, tsl], in_=x_ap[:, :, tsl])

            def stage1(ti):
                tsl = slice(ti * 512, (ti + 1) * 512)
                for mi in range(4):
                    ps = ps_mm.tile([128, 512], F32, tag="mm")
                    for ko in range(KO):
                        nc.tensor.matmul(
                            ps,
                            lhsT=wqk_sb[:, ko, mi * 128:(mi + 1) * 128],
                            rhs=xT_sb[:, ko, tsl],
                            start=(ko == 0), stop=(ko == KO - 1),
                        )
                    if mi < 2:
                        nc.vector.tensor_copy(out=qkT_sb[:, mi, tsl], in_=ps)
                    elif ROWPACK:
                        nc.scalar.copy(out=kT_sb[:, mi - 2, tsl], in_=ps)
                    else:
                        hp = (mi - 2) * 2
                        nc.scalar.copy(out=kT_sb[0:64, hp, tsl], in_=ps[0:64])
                        nc.scalar.copy(
                            out=kT_sb[64:128, hp + 1, tsl], in_=ps[64:128])
                for j in range(4):
                    t0 = ti * 512 + j * 128
                    ps = ps_mm.tile([128, 512], F32, tag="mm")
                    for ko in range(KO):
                        nc.tensor.matmul(
                            ps[:, 0:HPC * D],
                            lhsT=xT_sb[:, ko, t0:t0 + 128],
                            rhs=wv_sb[:, ko, :],
                            start=(ko == 0), stop=(ko == KO - 1),
                        )
                    nc.vector.tensor_copy(
                        out=vA_sb[:, ti * 4 + j, :, 0:D],
                        in_=ps[:, 0:HPC * D])

            def emit_S(qi):
                """S^T matmuls + exp + band mask; returns E^T sbuf tile
                laid out [128 k, par, ci, mi, q] (head h = 2*mi + par)."""
                qsl = slice(qi * 128, (qi + 1) * 128)
                cis = [1] if qi == 0 else [0, 1]
                psS = [ps_s.tile([128, 2, 2, 128], F32, tag=f"S{par}",
                                 name=f"psS{par}")
                       for par in range(2)]
                for ci in cis:
                    kt = qi - 1 + ci
                    ksl = slice(kt * 128, (kt + 1) * 128)
                    for mi in range(2):
                        for par in range(2):
                            if ROWPACK:
                                prows = slice(par * 64, par * 64 + 64)
                                lhsT = kT_sb[prows, mi, ksl]
                                rhs = qkT_sb[prows, mi, qsl]
                            else:
                                lhsT = kT_sb[:, 2 * mi + par, ksl]
                                rhs = qkT_sb[:, mi, qsl]
                            nc.tensor.matmul(
                                psS[par][:, ci, mi, :], lhsT=lhsT, rhs=rhs,
                                start=True, stop=True,
                            )
                esb = work.tile([128, 2, 2, 2, 128], BF16, tag="E")
                for par in range(2):
                    if qi == 0:
                        nc.scalar.activation(
                            out=esb[:, par, 1, :, :], in_=psS[par][:, 1, :, :],
                            func=mybir.ActivationFunctionType.Exp)
                    else:
                        nc.scalar.activation(
                            out=esb[:, par, :, :, :], in_=psS[par],
                            func=mybir.ActivationFunctionType.Exp)
                if qi == 0:
                    nc.gpsimd.memset(esb[:, :, 0, :, :], 0.0)
                # band mask -> zero.  esb free dims in the selected slice are
                # (par, mi, q); partition is k (local to the key tile).
                # ci=1 (kt==qi): keep iff q - k >= 0
                nc.gpsimd.affine_select(
                    out=esb[:, :, 1, :, :], in_=esb[:, :, 1, :, :],
                    compare_op=mybir.AluOpType.is_ge, fill=0.0,
                    base=0, channel_multiplier=-1,
                    pattern=[[0, 2], [0, 2], [1, 128]],
                )
                if qi > 0:
                    # ci=0 (kt==qi-1): keep iff k - q - 1 >= 0
                    nc.gpsimd.affine_select(
                        out=esb[:, :, 0, :, :], in_=esb[:, :, 0, :, :],
                        compare_op=mybir.AluOpType.is_ge, fill=0.0,
                        base=-1, channel_multiplier=1,
                        pattern=[[0, 2], [0, 2], [-1, 128]],
                    )
                return esb

            def emit_O(qi, esb):
                """[O'^T ; l] matmuls, then attT = O'^T / l."""
                qsl = slice(qi * 128, (qi + 1) * 128)
                cis = [1] if qi == 0 else [0, 1]
                # col blocks ordered [h0, h2, h1, h3] so each parity's pair
                # of heads is contiguous for the normalize ops below
                psO = ps_o.tile([128, HPC, 128], F32, tag="O")
                for h in range(HPC):
                    mi, par = h // 2, h % 2
                    blk = par * 2 + mi
                    for i, ci in enumerate(cis):
                        kt = qi - 1 + ci
                        nc.tensor.matmul(
                            psO[:, blk, :],
                            lhsT=vA_sb[:, kt, h, :],
                            rhs=esb[:, par, ci, mi, :],
                            start=(i == 0), stop=(i == len(cis) - 1),
                        )
                # stage l into SBUF first: the approx reciprocal's
                # BITWISE_NOT seed needs the IEEE bit pattern, which a PSUM
                # read does not reliably provide on hardware
                l_sb = rlp.tile([64, HPC, 128], F32, tag="lsb")
                nc.scalar.copy(out=l_sb, in_=psO[64:128, :, :])
                rl = rlp.tile([64, HPC, 128], F32, tag="rl")
                # l > 0 and well-scaled: far from the approx-fast edge cases;
                # ~18 correct bits vs the multi-pass exact reciprocal (5x cost)
                nc.vector.reciprocal_approx_fast(out=rl, in_=l_sb)
                for s in range(2):
                    nc.vector.tensor_tensor(
                        attT_sb[s * 64:s * 64 + 64, :, qsl],
                        psO[0:64, 2 * s:2 * s + 2, :],
                        rl[:, 2 * s:2 * s + 2, :],
                        mybir.AluOpType.mult,
                    )

            def stage3(qi):
                tsl = slice(qi * 128, (qi + 1) * 128)
                o_sb = outw.tile([128, E], BF16, tag="osb")
                for nh in range(2):
                    po = ps_mm.tile([128, 512], F32, tag="mm")
                    for j in range(2):
                        nc.tensor.matmul(
                            po,
                            lhsT=attT_sb[:, j, tsl],
                            rhs=wout_sb[:, j, nh * 512:(nh + 1) * 512],
                            start=(j == 0), stop=(j == 1),
                        )
                    if nh == 0:
                        nc.vector.tensor_copy(out=o_sb[:, 0:512], in_=po)
                    else:
                        nc.scalar.copy(out=o_sb[:, 512:1024], in_=po)
                nc.sync.dma_start(out=outp[tsl, :], in_=o_sb)

            # ---- software-pipelined main loop.  stage1 runs a full block
            # (512 tokens) ahead of attention so S never waits on fresh
            # casts; O lags S by one q-tile (hides exp+mask) and stage3 lags
            # by two (hides the l-copy/reciprocal/normalize chain). ----
            esbs = {}

            def pump(qi):
                esbs[qi] = emit_S(qi)
                if qi >= 1:
                    emit_O(qi - 1, esbs.pop(qi - 1))
                if qi >= 2:
                    stage3(qi - 2)

            for ti in range(NT512):
                stage1(ti)
                if ti > 0:
                    for qi in range(4 * (ti - 1), 4 * ti):
                        pump(qi)
            for qi in range(4 * (NT512 - 1), NQT):
                pump(qi)
            emit_O(NQT - 1, esbs.pop(NQT - 1))
            stage3(NQT - 2)
            stage3(NQT - 1)

    nc.finalize()
    return nc


_NC_CACHE = None


def _get_nc():
    global _NC_CACHE
    if _NC_CACHE is None:
        _NC_CACHE = build_bass()
    return _NC_CACHE


def make_in_maps(x, w_qkv, w_out):
    x = np.asarray(x, dtype=np.float32)
    w_qkv = np.asarray(w_qkv, dtype=np.float32)
    w_out = np.asarray(w_out, dtype=np.float32)
    bf = ml_dtypes.bfloat16
    in_maps = []
    for c in range(N_CORES):
        b = c // 4
        hs = (c % 4) * HPC
        rows = slice(hs * D, (hs + HPC) * D)
        wq = w_qkv[0 * E:, :][rows] * SCALE    # fold 1/sqrt(D) into q
        wk = w_qkv[1 * E:, :][rows]
        wvs = w_qkv[2 * E:, :][rows]
        in_maps.append({
            "xT": np.ascontiguousarray(x[b].T).astype(bf),
            "wqk": np.ascontiguousarray(
                np.concatenate([wq, wk], axis=0).T).astype(bf),
            "wv": np.ascontiguousarray(wvs.T).astype(bf),
            "wout": np.ascontiguousarray(w_out[:, rows].T).astype(bf),
        })
    return in_maps


def run(x, w_qkv, w_out, **spmd_kwargs):
    nc = _get_nc()
    in_maps = make_in_maps(x, w_qkv, w_out)
    res = run_bass_kernel_spmd(nc, in_maps, core_ids=list(range(N_CORES)),
                               **spmd_kwargs)
    outs = [r["outp"] for r in res.results]
    out = np.empty((B, T, E), dtype=np.float32)
    for b in range(B):
        acc = outs[4 * b].astype(np.float32)
        for c in range(4 * b + 1, 4 * b + 4):
            acc = acc + outs[c].astype(np.float32)
        out[b] = acc
    return out, res


def kernel(x, w_qkv, w_out):
    out, _ = run(x, w_qkv, w_out)
    return out


# revision 12
# speedup vs baseline: 1.5660x; 1.0115x over previous
"""Sliding-window (W=128) multi-head attention block for Trainium2, 8 cores.

Reference computation (B=2, T=2048, E=1024, H=16, D=64, W=128):
    qkv = x @ w_qkv.T ; split q,k,v ; heads ; att = softmax(mask(q k^T / 8)) v
    out = att_concat @ w_out.T

Sharding: data-parallel over B (2) x tensor-parallel over head groups (4),
so each of the 8 cores handles (one batch, 4 heads).  The output projection
is computed per-core against the 256 w_out columns belonging to its heads,
giving a partial [T, E] output (bf16); the host sums the 4 partials per
batch in f32.

Attention is computed in "transposed score" form to avoid PE transposes:
    S^T[k, q] = k^T.T @ q^T   (keys on partitions, d=64 contraction)
    E^T = exp(S^T)            (ACT, psum->sbuf bf16)
    mask -> 0 via affine_select on GpSimd (band structure is affine)
    [O'^T ; l] = [V | 1].T @ E^T  (l = softmax denominator, folded into the
                                   O matmul via ones-columns in the weights)
    attT = O'^T * (1/l)       (DVE reciprocal + multiply, psum in)
The two heads of a q^T/k^T pair chunk live at partitions 0:64 / 64:128 and
their K=64 S^T matmuls are row-tiled (tile_position auto-derived from the
base partition), so the pair runs concurrently on the PE array.

The 1/sqrt(D) scale is folded into the q weights on the host.
"""

import numpy as np
import ml_dtypes

import concourse.bass as bass
import concourse.bacc as bacc
import concourse.mybir as mybir
import concourse.tile as tile
from concourse.bass_utils import run_bass_kernel_spmd

B, T, E, H, W = 2, 2048, 1024, 16, 128
D = E // H            # 64
HPC = 4               # heads per core
N_CORES = 8
SCALE = 1.0 / float(np.sqrt(D))

BF16 = mybir.dt.bfloat16
F32 = mybir.dt.float32

KO = E // 128         # 8 contraction chunks
NQT = T // 128        # 16 query tiles
NT512 = T // 512      # 4 tiles for the projections

# Row-tiled K=64 S^T matmuls (pair-concurrent).  False falls back to
# zero-padded K=128 per-head kT (no partition-offset PE operands).
ROWPACK = True


def build_bass():
    nc = bacc.Bacc()
    xT = nc.declare_dram_parameter("xT", [E, T], BF16, isOutput=False)
    wqk = nc.declare_dram_parameter("wqk", [E, 2 * HPC * D], BF16, isOutput=False)
    wv = nc.declare_dram_parameter("wv", [E, HPC * D], BF16, isOutput=False)
    wout = nc.declare_dram_parameter("wout", [HPC * D, E], BF16, isOutput=False)
    outp = nc.declare_dram_parameter("outp", [T, E], BF16, isOutput=True)

    with tile.TileContext(nc) as tc:
        with (
            tc.tile_pool(name="persist", bufs=1) as persist,
            tc.tile_pool(name="work", bufs=3) as work,
            tc.tile_pool(name="rlp", bufs=2) as rlp,
            tc.tile_pool(name="outw", bufs=3) as outw,
            tc.tile_pool(name="ps_mm", bufs=2, space="PSUM") as ps_mm,
            tc.tile_pool(name="ps_s", bufs=2, space="PSUM") as ps_s,
            tc.tile_pool(name="ps_o", bufs=2, space="PSUM") as ps_o,
        ):
            # ---- persistent tiles ----
            wqk_sb = persist.tile([128, KO, 2 * HPC * D], BF16)
            wv_sb = persist.tile([128, KO, HPC * D], BF16)
            wout_sb = persist.tile([128, 2, E], BF16)
            xT_sb = persist.tile([128, KO, T], BF16)
            qkT_sb = persist.tile([128, 2, T], BF16)   # q^T pairs (scaled)
            if ROWPACK:
                kT_sb = persist.tile([128, 2, T], BF16)  # k^T pairs
            else:
                kT_sb = persist.tile([128, HPC, T], BF16)  # zero-padded k^T
                nc.vector.memset(kT_sb, 0.0)
            # V augmented with ones columns: [:, kt, h, 0:64]=V_h, 64:128=1
            vA_sb = persist.tile([128, NQT, HPC, 128], BF16)
            nc.gpsimd.memset(vA_sb[:, :, :, D:128], 1.0)
            attT_sb = persist.tile([128, 2, T], BF16)  # O^T, chunk j: heads 2j,2j+1

            # ---- PE warmup: ~16 dependency-free matmuls release the HAM
            # clock gate (K=4/8 -> 8/8 needs ~3.4us of sustained PE activity)
            # while the first input DMAs are still in flight ----
            wu_sb = persist.tile([128, 512], BF16)
            nc.vector.memset(wu_sb, 0.0)
            for wi in range(32):
                wu_ps = ps_o.tile([128, HPC, 128], F32, tag="O")
                nc.tensor.matmul(
                    wu_ps.rearrange("p a b -> p (a b)"), lhsT=wu_sb[:, 0:128],
                    rhs=wu_sb, start=True, stop=True)

            # ---- input DMAs (order matters: ti=0 needs wqk + x0 first;
            # split those by ko-halves so the first matmul chain can start
            # after ~1MB instead of 2MB) ----
            wqk_ap = wqk[:, :].rearrange("(ko p) m -> p ko m", p=128)
            x_ap = xT[:, :].rearrange("(ko p) t -> p ko t", p=128)
            nc.sync.dma_start(out=wqk_sb[:, 0:4], in_=wqk_ap[:, 0:4])
            nc.sync.dma_start(out=xT_sb[:, 0:4, 0:512], in_=x_ap[:, 0:4, 0:512])
            nc.sync.dma_start(out=wqk_sb[:, 4:8], in_=wqk_ap[:, 4:8])
            nc.sync.dma_start(out=xT_sb[:, 4:8, 0:512], in_=x_ap[:, 4:8, 0:512])
            nc.sync.dma_start(
                out=wv_sb, in_=wv[:, :].rearrange("(ko p) m -> p ko m", p=128))
            nc.sync.dma_start(
                out=wout_sb, in_=wout[:, :].rearrange("(c p) m -> p c m", p=128))
            for ti in range(1, NT512):
                tsl = slice(ti * 512, (ti + 1) * 512)
                nc.sync.dma_start(out=xT_sb[:, :, tsl], in_=x_ap[:, :, tsl])

            def stage1(ti):
                tsl = slice(ti * 512, (ti + 1) * 512)
                for mi in range(4):
                    ps = ps_mm.tile([128, 512], F32, tag="mm")
                    for ko in range(KO):
                        nc.tensor.matmul(
                            ps,
                            lhsT=wqk_sb[:, ko, mi * 128:(mi + 1) * 128],
                            rhs=xT_sb[:, ko, tsl],
                            start=(ko == 0), stop=(ko == KO - 1),
                        )
                    if mi < 2:
                        nc.vector.tensor_copy(out=qkT_sb[:, mi, tsl], in_=ps)
                    elif ROWPACK:
                        nc.scalar.copy(out=kT_sb[:, mi - 2, tsl], in_=ps)
                    else:
                        hp = (mi - 2) * 2
                        nc.scalar.copy(out=kT_sb[0:64, hp, tsl], in_=ps[0:64])
                        nc.scalar.copy(
                            out=kT_sb[64:128, hp + 1, tsl], in_=ps[64:128])
                for j in range(4):
                    t0 = ti * 512 + j * 128
                    ps = ps_mm.tile([128, 512], F32, tag="mm")
                    for ko in range(KO):
                        nc.tensor.matmul(
                            ps[:, 0:HPC * D],
                            lhsT=xT_sb[:, ko, t0:t0 + 128],
                            rhs=wv_sb[:, ko, :],
                            start=(ko == 0), stop=(ko == KO - 1),
                        )
                    nc.vector.tensor_copy(
                        out=vA_sb[:, ti * 4 + j, :, 0:D],
                        in_=ps[:, 0:HPC * D])

            def emit_S(qi):
                """S^T matmuls + exp + band mask; returns E^T sbuf tile
                laid out [128 k, par, ci, mi, q] (head h = 2*mi + par)."""
                qsl = slice(qi * 128, (qi + 1) * 128)
                cis = [1] if qi == 0 else [0, 1]
                psS = [ps_s.tile([128, 2, 2, 128], F32, tag=f"S{par}",
                                 name=f"psS{par}")
                       for par in range(2)]
                for ci in cis:
                    kt = qi - 1 + ci
                    ksl = slice(kt * 128, (kt + 1) * 128)
                    for mi in range(2):
                        for par in range(2):
                            if ROWPACK:
                                prows = slice(par * 64, par * 64 + 64)
                                lhsT = kT_sb[prows, mi, ksl]
                                rhs = qkT_sb[prows, mi, qsl]
                            else:
                                lhsT = kT_sb[:, 2 * mi + par, ksl]
                                rhs = qkT_sb[:, mi, qsl]
                            nc.tensor.matmul(
                                psS[par][:, ci, mi, :], lhsT=lhsT, rhs=rhs,
                                start=True, stop=True,
                            )
                esb = work.tile([128, 2, 2, 2, 128], BF16, tag="E")
                for par in range(2):
                    if qi == 0:
                        nc.scalar.activation(
                            out=esb[:, par, 1, :, :], in_=psS[par][:, 1, :, :],
                            func=mybir.ActivationFunctionType.Exp)
                    else:
                        nc.scalar.activation(
                            out=esb[:, par, :, :, :], in_=psS[par],
                            func=mybir.ActivationFunctionType.Exp)
                if qi == 0:
                    nc.gpsimd.memset(esb[:, :, 0, :, :], 0.0)
                # band mask -> zero.  esb free dims in the selected slice are
                # (par, mi, q); partition is k (local to the key tile).
                # ci=1 (kt==qi): keep iff q - k >= 0
                nc.gpsimd.affine_select(
                    out=esb[:, :, 1, :, :], in_=esb[:, :, 1, :, :],
                    compare_op=mybir.AluOpType.is_ge, fill=0.0,
                    base=0, channel_multiplier=-1,
                    pattern=[[0, 2], [0, 2], [1, 128]],
                )
                if qi > 0:
                    # ci=0 (kt==qi-1): keep iff k - q - 1 >= 0
                    nc.gpsimd.affine_select(
                        out=esb[:, :, 0, :, :], in_=esb[:, :, 0, :, :],
                        compare_op=mybir.AluOpType.is_ge, fill=0.0,
                        base=-1, channel_multiplier=1,
                        pattern=[[0, 2], [0, 2], [-1, 128]],
                    )
                return esb

            def emit_O(qi, esb):
                """[O'^T ; l] matmuls, then attT = O'^T / l."""
                qsl = slice(qi * 128, (qi + 1) * 128)
                cis = [1] if qi == 0 else [0, 1]
                # col blocks ordered [h0, h2, h1, h3] so each parity's pair
                # of heads is contiguous for the normalize ops below
                psO = ps_o.tile([128, HPC, 128], F32, tag="O")
                for h in range(HPC):
                    mi, par = h // 2, h % 2
                    blk = par * 2 + mi
                    for i, ci in enumerate(cis):
                        kt = qi - 1 + ci
                        nc.tensor.matmul(
                            psO[:, blk, :],
                            lhsT=vA_sb[:, kt, h, :],
                            rhs=esb[:, par, ci, mi, :],
                            start=(i == 0), stop=(i == len(cis) - 1),
                        )
                # stage l into SBUF first: the approx reciprocal's
                # BITWISE_NOT seed needs the IEEE bit pattern, which a PSUM
                # read does not reliably provide on hardware
                l_sb = rlp.tile([64, HPC, 128], F32, tag="lsb")
                nc.scalar.copy(out=l_sb, in_=psO[64:128, :, :])
                rl = rlp.tile([64, HPC, 128], F32, tag="rl")
                # l > 0 and well-scaled: far from the approx-fast edge cases;
                # ~18 correct bits vs the multi-pass exact reciprocal (5x cost)
                nc.vector.reciprocal_approx_fast(out=rl, in_=l_sb)
                for s in range(2):
                    nc.vector.tensor_tensor(
                        attT_sb[s * 64:s * 64 + 64, :, qsl],
                        psO[0:64, 2 * s:2 * s + 2, :],
                        rl[:, 2 * s:2 * s + 2, :],
                        mybir.AluOpType.mult,
                    )

            def stage3(qi):
                tsl = slice(qi * 128, (qi + 1) * 128)
                o_sb = outw.tile([128, E], BF16, tag="osb")
                for nh in range(2):
                    po = ps_mm.tile([128, 512], F32, tag="mm")
                    for j in range(2):
                        nc.tensor.matmul(
                            po,
                            lhsT=attT_sb[:, j, tsl],
                            rhs=wout_sb[:, j, nh * 512:(nh + 1) * 512],
                            start=(j == 0), stop=(j == 1),
                        )
                    if nh == 0:
                        nc.vector.tensor_copy(out=o_sb[:, 0:512], in_=po)
                    else:
                        nc.scalar.copy(out=o_sb[:, 512:1024], in_=po)
                nc.sync.dma_start(out=outp[tsl, :], in_=o_sb)

            # ---- software-pipelined main loop.  stage1 runs a full block
            # (512 tokens) ahead of attention so S never waits on fresh
            # casts; O lags S by one q-tile (hides exp+mask) and stage3 lags
            # by two (hides the l-copy/reciprocal/normalize chain). ----
            esbs = {}

            def pump(qi):
                esbs[qi] = emit_S(qi)
                if qi >= 1:
                    emit_O(qi - 1, esbs.pop(qi - 1))
                if qi >= 2:
                    stage3(qi - 2)

            for ti in range(NT512):
                stage1(ti)
                if ti > 0:
                    for qi in range(4 * (ti - 1), 4 * ti):
                        pump(qi)
            for qi in range(4 * (NT512 - 1), NQT):
                pump(qi)
            emit_O(NQT - 1, esbs.pop(NQT - 1))
            stage3(NQT - 2)
            stage3(NQT - 1)

    nc.finalize()
    return nc


_NC_CACHE = None


def _get_nc():
    global _NC_CACHE
    if _NC_CACHE is None:
        _NC_CACHE = build_bass()
    return _NC_CACHE


def make_in_maps(x, w_qkv, w_out):
    x = np.asarray(x, dtype=np.float32)
    w_qkv = np.asarray(w_qkv, dtype=np.float32)
    w_out = np.asarray(w_out, dtype=np.float32)
    bf = ml_dtypes.bfloat16
    in_maps = []
    for c in range(N_CORES):
        b = c // 4
        hs = (c % 4) * HPC
        rows = slice(hs * D, (hs + HPC) * D)
        wq = w_qkv[0 * E:, :][rows] * SCALE    # fold 1/sqrt(D) into q
        wk = w_qkv[1 * E:, :][rows]
        wvs = w_qkv[2 * E:, :][rows]
        in_maps.append({
            "xT": np.ascontiguousarray(x[b].T).astype(bf),
            "wqk": np.ascontiguousarray(
                np.concatenate([wq, wk], axis=0).T).astype(bf),
            "wv": np.ascontiguousarray(wvs.T).astype(bf),
            "wout": np.ascontiguousarray(w_out[:, rows].T).astype(bf),
        })
    return in_maps


def run(x, w_qkv, w_out, **spmd_kwargs):
    nc = _get_nc()
    in_maps = make_in_maps(x, w_qkv, w_out)
    res = run_bass_kernel_spmd(nc, in_maps, core_ids=list(range(N_CORES)),
                               **spmd_kwargs)
    outs = [r["outp"] for r in res.results]
    out = np.empty((B, T, E), dtype=np.float32)
    for b in range(B):
        acc = outs[4 * b].astype(np.float32)
        for c in range(4 * b + 1, 4 * b + 4):
            acc = acc + outs[c].astype(np.float32)
        out[b] = acc
    return out, res


def kernel(x, w_qkv, w_out):
    out, _ = run(x, w_qkv, w_out)
    return out


# revision 15
# speedup vs baseline: 1.6471x; 1.0518x over previous
"""Sliding-window (W=128) multi-head attention block for Trainium2, 8 cores.

Reference computation (B=2, T=2048, E=1024, H=16, D=64, W=128):
    qkv = x @ w_qkv.T ; split q,k,v ; heads ; att = softmax(mask(q k^T / 8)) v
    out = att_concat @ w_out.T

Sharding: data-parallel over B (2) x tensor-parallel over head groups (4),
so each of the 8 cores handles (one batch, 4 heads).  The output projection
is computed per-core against the 256 w_out columns belonging to its heads,
giving a partial [T, E] output (bf16); the host sums the 4 partials per
batch in f32.

Attention is computed in "transposed score" form to avoid PE transposes:
    S^T[k, q] = k^T.T @ q^T   (keys on partitions, d=64 contraction)
    E^T = exp(S^T)            (ACT, psum->sbuf bf16)
    mask -> 0 via affine_select on GpSimd (band structure is affine)
    [O'^T ; l] = [V | 1].T @ E^T  (l = softmax denominator, folded into the
                                   O matmul via ones-columns in the weights)
    attT = O'^T * (1/l)       (DVE reciprocal + multiply, psum in)
The two heads of a q^T/k^T pair chunk live at partitions 0:64 / 64:128 and
their K=64 S^T matmuls are row-tiled (tile_position auto-derived from the
base partition), so the pair runs concurrently on the PE array.

The 1/sqrt(D) scale is folded into the q weights on the host.
"""

import numpy as np
import ml_dtypes

import concourse.bass as bass
import concourse.bacc as bacc
import concourse.mybir as mybir
import concourse.tile as tile
from concourse.bass_utils import run_bass_kernel_spmd

B, T, E, H, W = 2, 2048, 1024, 16, 128
D = E // H            # 64
HPC = 4               # heads per core
N_CORES = 8
SCALE = 1.0 / float(np.sqrt(D))

BF16 = mybir.dt.bfloat16
F32 = mybir.dt.float32

KO = E // 128         # 8 contraction chunks
NQT = T // 128        # 16 query tiles
NT512 = T // 512      # 4 tiles for the projections

# Row-tiled K=64 S^T matmuls (pair-concurrent).  False falls back to
# zero-padded K=128 per-head kT (no partition-offset PE operands).
ROWPACK = True


def build_bass():
    nc = bacc.Bacc()
    xT = nc.declare_dram_parameter("xT", [E, T], BF16, isOutput=False)
    wqk = nc.declare_dram_parameter("wqk", [E, 2 * HPC * D], BF16, isOutput=False)
    wv = nc.declare_dram_parameter("wv", [E, HPC * D], BF16, isOutput=False)
    wout = nc.declare_dram_parameter("wout", [HPC * D, E], BF16, isOutput=False)
    outp = nc.declare_dram_parameter("outp", [T, E], BF16, isOutput=True)

    with tile.TileContext(nc) as tc:
        with (
            tc.tile_pool(name="persist", bufs=1) as persist,
            tc.tile_pool(name="work", bufs=3) as work,
            tc.tile_pool(name="rlp", bufs=2) as rlp,
            tc.tile_pool(name="outw", bufs=3) as outw,
            tc.tile_pool(name="ps_mm", bufs=2, space="PSUM") as ps_mm,
            tc.tile_pool(name="ps_s", bufs=2, space="PSUM") as ps_s,
            tc.tile_pool(name="ps_o", bufs=2, space="PSUM") as ps_o,
        ):
            # ---- persistent tiles ----
            wqk_sb = persist.tile([128, KO, 2 * HPC * D], BF16)
            wv_sb = persist.tile([128, KO, HPC * D], BF16)
            wout_sb = persist.tile([128, 2, E], BF16)
            xT_sb = persist.tile([128, KO, T], BF16)
            qkT_sb = persist.tile([128, 2, T], BF16)   # q^T pairs (scaled)
            if ROWPACK:
                kT_sb = persist.tile([128, 2, T], BF16)  # k^T pairs
            else:
                kT_sb = persist.tile([128, HPC, T], BF16)  # zero-padded k^T
                nc.vector.memset(kT_sb, 0.0)
            # V augmented with ones columns: [:, kt, h, 0:64]=V_h, 64:128=1
            vA_sb = persist.tile([128, NQT, HPC, 128], BF16)
            nc.gpsimd.memset(vA_sb[:, :, :, D:128], 1.0)
            attT_sb = persist.tile([128, 2, T], BF16)  # O^T, chunk j: heads 2j,2j+1

            # ---- PE warmup: ~16 dependency-free matmuls release the HAM
            # clock gate (K=4/8 -> 8/8 needs ~3.4us of sustained PE activity)
            # while the first input DMAs are still in flight ----
            wu_sb = persist.tile([128, 512], BF16)
            nc.vector.memset(wu_sb, 0.0)
            for wi in range(20):
                wu_ps = ps_o.tile([128, HPC, 128], F32, tag="O")
                nc.tensor.matmul(
                    wu_ps.rearrange("p a b -> p (a b)"), lhsT=wu_sb[:, 0:128],
                    rhs=wu_sb, start=True, stop=True)

            # ---- input DMAs (order matters: ti=0 needs wqk + x0 first;
            # split those by ko-halves so the first matmul chain can start
            # after ~1MB instead of 2MB) ----
            wqk_ap = wqk[:, :].rearrange("(ko p) m -> p ko m", p=128)
            x_ap = xT[:, :].rearrange("(ko p) t -> p ko t", p=128)
            nc.sync.dma_start(out=wqk_sb[:, 0:4], in_=wqk_ap[:, 0:4])
            nc.sync.dma_start(out=xT_sb[:, 0:4, 0:512], in_=x_ap[:, 0:4, 0:512])
            nc.sync.dma_start(out=wqk_sb[:, 4:8], in_=wqk_ap[:, 4:8])
            nc.sync.dma_start(out=xT_sb[:, 4:8, 0:512], in_=x_ap[:, 4:8, 0:512])
            nc.sync.dma_start(
                out=wv_sb, in_=wv[:, :].rearrange("(ko p) m -> p ko m", p=128))
            nc.sync.dma_start(
                out=wout_sb, in_=wout[:, :].rearrange("(c p) m -> p c m", p=128))
            for ti in range(1, NT512):
                tsl = slice(ti * 512, (ti + 1) * 512)
                nc.sync.dma_start(out=xT_sb[:, :, tsl], in_=x_ap[:, :, tsl])

            def qk_chunk(ti, mi):
                tsl = slice(ti * 512, (ti + 1) * 512)
                ps = ps_mm.tile([128, 512], F32, tag="mm")
                for ko in range(KO):
                    nc.tensor.matmul(
                        ps,
                        lhsT=wqk_sb[:, ko, mi * 128:(mi + 1) * 128],
                        rhs=xT_sb[:, ko, tsl],
                        start=(ko == 0), stop=(ko == KO - 1),
                    )
                if mi < 2:
                    nc.vector.tensor_copy(out=qkT_sb[:, mi, tsl], in_=ps)
                elif ROWPACK:
                    nc.scalar.copy(out=kT_sb[:, mi - 2, tsl], in_=ps)
                else:
                    hp = (mi - 2) * 2
                    nc.scalar.copy(out=kT_sb[0:64, hp, tsl], in_=ps[0:64])
                    nc.scalar.copy(
                        out=kT_sb[64:128, hp + 1, tsl], in_=ps[64:128])

            def v_chunk(ti, j):
                t0 = ti * 512 + j * 128
                ps = ps_mm.tile([128, 512], F32, tag="mm")
                for ko in range(KO):
                    nc.tensor.matmul(
                        ps[:, 0:HPC * D],
                        lhsT=xT_sb[:, ko, t0:t0 + 128],
                        rhs=wv_sb[:, ko, :],
                        start=(ko == 0), stop=(ko == KO - 1),
                    )
                nc.vector.tensor_copy(
                    out=vA_sb[:, ti * 4 + j, :, 0:D],
                    in_=ps[:, 0:HPC * D])

            def emit_S(qi):
                """S^T matmuls + exp + band mask; returns E^T sbuf tile
                laid out [128 k, par, ci, mi, q] (head h = 2*mi + par)."""
                qsl = slice(qi * 128, (qi + 1) * 128)
                cis = [1] if qi == 0 else [0, 1]
                psS = [ps_s.tile([128, 2, 2, 128], F32, tag=f"S{par}",
                                 name=f"psS{par}")
                       for par in range(2)]
                for ci in cis:
                    kt = qi - 1 + ci
                    ksl = slice(kt * 128, (kt + 1) * 128)
                    for mi in range(2):
                        for par in range(2):
                            if ROWPACK:
                                prows = slice(par * 64, par * 64 + 64)
                                lhsT = kT_sb[prows, mi, ksl]
                                rhs = qkT_sb[prows, mi, qsl]
                            else:
                                lhsT = kT_sb[:, 2 * mi + par, ksl]
                                rhs = qkT_sb[:, mi, qsl]
                            nc.tensor.matmul(
                                psS[par][:, ci, mi, :], lhsT=lhsT, rhs=rhs,
                                start=True, stop=True,
                            )
                esb = work.tile([128, 2, 2, 2, 128], BF16, tag="E")
                for par in range(2):
                    if qi == 0:
                        nc.scalar.activation(
                            out=esb[:, par, 1, :, :], in_=psS[par][:, 1, :, :],
                            func=mybir.ActivationFunctionType.Exp)
                    else:
                        nc.scalar.activation(
                            out=esb[:, par, :, :, :], in_=psS[par],
                            func=mybir.ActivationFunctionType.Exp)
                if qi == 0:
                    nc.gpsimd.memset(esb[:, :, 0, :, :], 0.0)
                # band mask -> zero.  esb free dims in the selected slice are
                # (par, mi, q); partition is k (local to the key tile).
                # ci=1 (kt==qi): keep iff q - k >= 0
                nc.gpsimd.affine_select(
                    out=esb[:, :, 1, :, :], in_=esb[:, :, 1, :, :],
                    compare_op=mybir.AluOpType.is_ge, fill=0.0,
                    base=0, channel_multiplier=-1,
                    pattern=[[0, 2], [0, 2], [1, 128]],
                )
                if qi > 0:
                    # ci=0 (kt==qi-1): keep iff k - q - 1 >= 0
                    nc.gpsimd.affine_select(
                        out=esb[:, :, 0, :, :], in_=esb[:, :, 0, :, :],
                        compare_op=mybir.AluOpType.is_ge, fill=0.0,
                        base=-1, channel_multiplier=1,
                        pattern=[[0, 2], [0, 2], [-1, 128]],
                    )
                return esb

            def emit_O(qi, esb):
                """[O'^T ; l] matmuls, then attT = O'^T / l."""
                qsl = slice(qi * 128, (qi + 1) * 128)
                cis = [1] if qi == 0 else [0, 1]
                # col blocks ordered [h0, h2, h1, h3] so each parity's pair
                # of heads is contiguous for the normalize ops below
                psO = ps_o.tile([128, HPC, 128], F32, tag="O")
                for h in range(HPC):
                    mi, par = h // 2, h % 2
                    blk = par * 2 + mi
                    for i, ci in enumerate(cis):
                        kt = qi - 1 + ci
                        nc.tensor.matmul(
                            psO[:, blk, :],
                            lhsT=vA_sb[:, kt, h, :],
                            rhs=esb[:, par, ci, mi, :],
                            start=(i == 0), stop=(i == len(cis) - 1),
                        )
                # stage l into SBUF first: the approx reciprocal's
                # BITWISE_NOT seed needs the IEEE bit pattern, which a PSUM
                # read does not reliably provide on hardware
                l_sb = rlp.tile([64, HPC, 128], F32, tag="lsb")
                nc.scalar.copy(out=l_sb, in_=psO[64:128, :, :])
                rl = rlp.tile([64, HPC, 128], F32, tag="rl")
                # l > 0 and well-scaled: far from the approx-fast edge cases;
                # ~18 correct bits vs the multi-pass exact reciprocal (5x cost)
                nc.vector.reciprocal_approx_fast(out=rl, in_=l_sb)
                for s in range(2):
                    nc.vector.tensor_tensor(
                        attT_sb[s * 64:s * 64 + 64, :, qsl],
                        psO[0:64, 2 * s:2 * s + 2, :],
                        rl[:, 2 * s:2 * s + 2, :],
                        mybir.AluOpType.mult,
                    )

            def stage3(qi):
                tsl = slice(qi * 128, (qi + 1) * 128)
                o_sb = outw.tile([128, E], BF16, tag="osb")
                for nh in range(2):
                    po = ps_mm.tile([128, 512], F32, tag="mm")
                    for j in range(2):
                        nc.tensor.matmul(
                            po,
                            lhsT=attT_sb[:, j, tsl],
                            rhs=wout_sb[:, j, nh * 512:(nh + 1) * 512],
                            start=(j == 0), stop=(j == 1),
                        )
                    if nh == 0:
                        nc.vector.tensor_copy(out=o_sb[:, 0:512], in_=po)
                    else:
                        nc.scalar.copy(out=o_sb[:, 512:1024], in_=po)
                nc.sync.dma_start(out=outp[tsl, :], in_=o_sb)

            # ---- software-pipelined main loop.  stage1 runs a full block
            # (512 tokens) ahead of attention so S never waits on fresh
            # casts; O lags S by one q-tile (hides exp+mask) and stage3 lags
            # by two (hides the l-copy/reciprocal/normalize chain). ----
            esbs = {}

            def pump(qi):
                esbs[qi] = emit_S(qi)
                if qi >= 1:
                    emit_O(qi - 1, esbs.pop(qi - 1))
                if qi >= 2:
                    stage3(qi - 2)

            for ti in range(NT512):
                chunks = ([lambda mi=mi: qk_chunk(ti, mi) for mi in range(4)]
                          + [lambda j=j: v_chunk(ti, j) for j in range(4)])
                if ti == 0:
                    for c in chunks:
                        c()
                else:
                    # spread the prev block's attention pumps through this
                    # block's projection chunks so ACT/DVE/GpSimd work is
                    # evenly distributed under the PE's dense matmul stream
                    for i in range(4):
                        chunks[2 * i]()
                        chunks[2 * i + 1]()
                        pump(4 * (ti - 1) + i)
            for qi in range(4 * (NT512 - 1), NQT):
                pump(qi)
            emit_O(NQT - 1, esbs.pop(NQT - 1))
            stage3(NQT - 2)
            stage3(NQT - 1)

    nc.finalize()
    return nc


_NC_CACHE = None


def _get_nc():
    global _NC_CACHE
    if _NC_CACHE is None:
        _NC_CACHE = build_bass()
    return _NC_CACHE


def make_in_maps(x, w_qkv, w_out):
    x = np.asarray(x, dtype=np.float32)
    w_qkv = np.asarray(w_qkv, dtype=np.float32)
    w_out = np.asarray(w_out, dtype=np.float32)
    bf = ml_dtypes.bfloat16
    in_maps = []
    for c in range(N_CORES):
        b = c // 4
        hs = (c % 4) * HPC
        rows = slice(hs * D, (hs + HPC) * D)
        wq = w_qkv[0 * E:, :][rows] * SCALE    # fold 1/sqrt(D) into q
        wk = w_qkv[1 * E:, :][rows]
        wvs = w_qkv[2 * E:, :][rows]
        in_maps.append({
            "xT": np.ascontiguousarray(x[b].T).astype(bf),
            "wqk": np.ascontiguousarray(
                np.concatenate([wq, wk], axis=0).T).astype(bf),
            "wv": np.ascontiguousarray(wvs.T).astype(bf),
            "wout": np.ascontiguousarray(w_out[:, rows].T).astype(bf),
        })
    return in_maps


def run(x, w_qkv, w_out, **spmd_kwargs):
    nc = _get_nc()
    in_maps = make_in_maps(x, w_qkv, w_out)
    res = run_bass_kernel_spmd(nc, in_maps, core_ids=list(range(N_CORES)),
                               **spmd_kwargs)
    outs = [r["outp"] for r in res.results]
    out = np.empty((B, T, E), dtype=np.float32)
    for b in range(B):
        acc = outs[4 * b].astype(np.float32)
        for c in range(4 * b + 1, 4 * b + 4):
            acc = acc + outs[c].astype(np.float32)
        out[b] = acc
    return out, res


def kernel(x, w_qkv, w_out):
    out, _ = run(x, w_qkv, w_out)
    return out
